# revision 1
# baseline (speedup 1.0000x reference)
"""Multi-head self-attention (B=4, T=2048, D=1024, H=16) on 8 NeuronCores.

Sharding: batch x head-group. Core c handles batch b = c//2 and head group
g = c%2 (8 heads of 64 dims each). Host pre-transposes x and slices/
transposes the weights; each core computes its 8 heads' attention and a
partial output projection; host sums the two partials per batch and adds bo.

Per-core pipeline (matmuls on TensorE, exp on ScalarE):
  v    = x @ WvT_aug + bv       (bf16, augmented with a ones column per head
                                 so attn@V also accumulates softmax sums)
  qT/kT = (W x^T + b), all 4 head-pairs in one x pass   (bf16)
  scoresT[k,q] = kT^T qT / 8    (two heads row-tiled concurrently, K=64)
  e = exp(scoresT)              (ScalarE, bf16 out)
  ctxT[dh,q] += (v|1)^T e       (bf16; row 64 = softmax denominator S)
  ctxT /= S                     (reciprocal + ones-matmul partition bcast, f32r)
  out_partial = ctxT^T WoT      (f32r for final precision)

HW-calibrated dtype costs (N=512 moving dim): bf16 1 cyc/row, fp32r 2,
fp32 4. Row-tiled K=64 pairs run concurrently (~181 ns/matmul measured).
"""

from contextlib import ExitStack

import numpy as np
import ml_dtypes

import concourse.bass as bass
import concourse.mybir as mybir
import concourse.tile as tile
from concourse import bacc
from concourse.bass_utils import run_bass_kernel_spmd

F32 = mybir.dt.float32
F32R = mybir.dt.float32r
BF16 = mybir.dt.bfloat16
EXP = mybir.ActivationFunctionType.Exp

B, T, D = 4, 2048, 1024
H, DH = 16, 64
G = 512            # head-group width (8 heads x 64)
GH = 8             # heads per group
P = 128
DK = D // P        # 8 contraction k-tiles for D
NQT = T // P       # 16 q/t tiles of 128
NQC = T // 512     # 4 q chunks of 512
NKT = T // P       # 16 key tiles of 128
VW = GH * (DH + 1)   # 520: v free width incl. ones columns
VC = VW // 2       # 260: v projection N-chunk (psum bank limit 512 fp32)


def r(ap):
    return ap.bitcast(F32R)


def emit_body(tc, io, phases=(1, 2, 3, 4)):
    nc = tc.nc
    xT, wq, wk, wv, wo, bq, bk, bv, out = (
        io["xT"], io["wq"], io["wk"], io["wv"], io["wo"],
        io["bq"], io["bk"], io["bv"], io["out"])

    xT_r = xT.rearrange("(i p) t -> p i t", p=P)     # [128, 8, 2048] bf16

    with ExitStack() as ctx:
        E = ctx.enter_context
        constp = E(tc.tile_pool(name="const", bufs=1))
        wqkp = E(tc.tile_pool(name="wqk", bufs=1))
        wvop = E(tc.tile_pool(name="wvo", bufs=1))
        vp = E(tc.tile_pool(name="vsb", bufs=1))
        ctxp = E(tc.tile_pool(name="ctxsb", bufs=1))
        qkp = E(tc.tile_pool(name="qksb", bufs=1))
        xs_p = E(tc.tile_pool(name="xs", bufs=6))
        ep = E(tc.tile_pool(name="e", bufs=8))
        rp = E(tc.tile_pool(name="recip", bufs=4))

        # ---- constants / weights resident in SBUF ----
        ones = constp.tile([P, 64], F32, name="ones")
        nc.sync.dma_start(r(ones[:]), r(io["ones"]))
        bq_sb = constp.tile([P, 4], F32, name="bq_sb")
        nc.sync.dma_start(bq_sb[:], bq[:])
        bk_sb = constp.tile([P, 4], F32, name="bk_sb")
        nc.sync.dma_start(bk_sb[:], bk[:])
        bv_sb = constp.tile([P, VW], F32, name="bv_sb")
        nc.sync.dma_start(bv_sb[:], bv[:])

        wq_sb = wqkp.tile([P, DK, G], BF16, name="wq_sb")
        nc.sync.dma_start(wq_sb[:], wq.rearrange("(i p) m -> p i m", p=P))
        wk_sb = wqkp.tile([P, DK, G], BF16, name="wk_sb")
        nc.sync.dma_start(wk_sb[:], wk.rearrange("(i p) m -> p i m", p=P))
        wv_sb = wvop.tile([P, DK, VW], BF16, name="wv_sb")
        nc.sync.dma_start(wv_sb[:], wv.rearrange("(i p) m -> p i m", p=P))
        wo_sb = wvop.tile([P, 4, D], F32, name="wo_sb")
        nc.sync.dma_start(r(wo_sb[:]), r(wo.rearrange("(i p) m -> p i m", p=P)))

        v_sb = vp.tile([P, NQT, VW], BF16, name="v_sb")
        # per-q-group ctx tiles: out-proj on q-tiles 0..7 only gates on
        # the first q-group's attention writes, overlapping the tail of
        # attention with the head of the output projection
        ctx_t = [ctxp.tile([P, 4, T // 2], F32, name=f"ctx_sb{g}")
                 for g in range(2)]
        # per-pair tiles (not one [P,4,T] tensor) so Tile's per-tensor
        # dependency gating lets attention on pair j start as soon as pair
        # j's projections are evicted, overlapping phase 2 of later pairs
        qT_t = [qkp.tile([P, T], BF16, name=f"qT_sb{j}") for j in range(4)]
        kT_t = [qkp.tile([P, T], BF16, name=f"kT_sb{j}") for j in range(4)]

        # ---- phase 1: v projection (all 8 heads), natural [t, hd] layout ----
        if 1 not in phases:
            nc.vector.memset(v_sb[:], 0.0)
        else:
         with tc.tile_pool(name="psv", bufs=8, space="PSUM") as ps_v:
            for tg in range(4):                  # groups of 4 t-tiles (512 t)
                v_ps = [[ps_v.tile([P, VC], F32, tag="vps",
                                   name=f"vps_{tg}_{ti}_{vc}")
                         for vc in range(2)] for ti in range(4)]
                for dk in range(DK):
                    xs = xs_p.tile([P, 512], BF16, tag="xs")
                    nc.sync.dma_start(
                        xs[:], xT_r[:, dk, tg * 512:(tg + 1) * 512])
                    for ti in range(4):
                        for vc in range(2):
                            nc.tensor.matmul(
                                v_ps[ti][vc][:],
                                xs[:, ti * P:(ti + 1) * P],
                                wv_sb[:, dk, vc * VC:(vc + 1) * VC],
                                start=(dk == 0), stop=(dk == DK - 1))
                for ti in range(4):
                    for vc in range(2):
                        nc.vector.tensor_add(
                            v_sb[:, tg * 4 + ti, vc * VC:(vc + 1) * VC],
                            v_ps[ti][vc][:], bv_sb[:, vc * VC:(vc + 1) * VC])

        # ---- phase 2: qT/kT for all 4 head-pairs in one x pass ----
        if 2 not in phases:
            for j in range(4):
                nc.vector.memset(qT_t[j][:], 0.0)
                nc.vector.memset(kT_t[j][:], 0.0)
        else:
         with tc.tile_pool(name="psqk", bufs=8, space="PSUM") as ps_qk:
            for qc in range(NQC):
                qs = slice(qc * 512, (qc + 1) * 512)
                q_ps = [ps_qk.tile([P, 512], F32, tag="qk", name=f"qps{qc}_{j}")
                        for j in range(4)]
                k_ps = [ps_qk.tile([P, 512], F32, tag="qk", name=f"kps{qc}_{j}")
                        for j in range(4)]
                for dk in range(DK):
                    xs = xs_p.tile([P, 512], BF16, tag="xs")
                    nc.sync.dma_start(xs[:], xT_r[:, dk, qs])
                    for j in range(4):
                        nc.tensor.matmul(
                            q_ps[j][:], wq_sb[:, dk, j * P:(j + 1) * P],
                            xs[:], start=(dk == 0), stop=(dk == DK - 1))
                        nc.tensor.matmul(
                            k_ps[j][:], wk_sb[:, dk, j * P:(j + 1) * P],
                            xs[:], start=(dk == 0), stop=(dk == DK - 1))
                for j in range(4):
                    # scores scale 1/sqrt(dh)=1/8 folded into q here so the
                    # exp activation runs with scale=1
                    nc.vector.tensor_scalar(
                        qT_t[j][:, qs], q_ps[j][:], bq_sb[:, j:j + 1], 0.125,
                        mybir.AluOpType.add, mybir.AluOpType.mult)
                    nc.vector.tensor_scalar_add(
                        kT_t[j][:, qs], k_ps[j][:], bk_sb[:, j:j + 1])

        # ---- phase 3: attention per head-pair ----
        # s tiles are [128, 1024] (2 banks, filled by two N=512 matmuls) so
        # exp runs as one [128,1024] activation. ctx accumulators are 4x
        # [65,512] banks per (j, qg). Raw ctx + denominator are evicted to
        # SBUF immediately so the banks free up; normalization (reciprocal,
        # ones-matmul broadcast, multiply) runs off-PSUM.
        if 3 not in phases:
            for g in range(2):
                nc.vector.memset(ctx_t[g][:], 0.0)
        else:
         with tc.tile_pool(name="pss", bufs=2, space="PSUM") as ps_s, \
             tc.tile_pool(name="psctx", bufs=4, space="PSUM") as ps_ctx, \
             tc.tile_pool(name="ctxu", bufs=6) as cup:
            for j in range(4):
                for qg in range(2):              # q groups of 1024
                    qsl = [slice(qg * 1024 + c * 512, qg * 1024 + (c + 1) * 512)
                           for c in range(2)]
                    ctxs = [[ps_ctx.tile([P, 512], F32, tag="ctx",
                                         name=f"ctx{j}_{qg}_{ab}_{c}")
                             for c in range(2)] for ab in range(2)]
                    # one-iteration software pipeline: emit kt's scores
                    # and exp, but kt-1's ctx matmuls, so the in-order PE
                    # queue never blocks behind the current exp (PE and
                    # ScalarE would otherwise fully serialize, measured
                    # 4.3us/kt vs 2.4us/kt pipelined).
                    def ctx_mms(eA, eB, kt):
                        for c in range(2):
                            nc.tensor.matmul(
                                ctxs[0][c][:65],
                                v_sb[:, kt, j * 130:j * 130 + 65],
                                eA[:, c * 512:(c + 1) * 512],
                                start=(kt == 0), stop=(kt == NKT - 1))
                            nc.tensor.matmul(
                                ctxs[1][c][:65],
                                v_sb[:, kt, j * 130 + 65:j * 130 + 130],
                                eB[:, c * 512:(c + 1) * 512],
                                start=(kt == 0), stop=(kt == NKT - 1))
                    prev = None
                    for kt in range(NKT):
                        ks = slice(kt * P, (kt + 1) * P)
                        sA = ps_s.tile([P, 1024], F32, tag="s")
                        sB = ps_s.tile([P, 1024], F32, tag="s")
                        # warm-keeper matmuls: discarded (the real sA matmul
                        # below also has start=True, overwriting them), but
                        # they keep TensorE dense through the exp-wait gap --
                        # idle gaps re-throttle the PE clock (HAM) and were
                        # measured to triple the per-iteration time.
                        for _ in range(4):
                            nc.tensor.matmul(
                                sA[:, 0:512], kT_t[j][0:64, ks],
                                qT_t[j][0:64, qsl[0]], start=True, stop=True,
                                tile_position=(0, 0))
                        for c in range(2):
                            nc.tensor.matmul(
                                sA[:, c * 512:(c + 1) * 512],
                                kT_t[j][0:64, ks], qT_t[j][0:64, qsl[c]],
                                start=True, stop=True, tile_position=(0, 0))
                            nc.tensor.matmul(
                                sB[:, c * 512:(c + 1) * 512],
                                kT_t[j][64:128, ks], qT_t[j][64:128, qsl[c]],
                                start=True, stop=True, tile_position=(64, 0))
                        eA = ep.tile([P, 1024], BF16, tag="e")
                        eB = ep.tile([P, 1024], BF16, tag="e")
                        nc.scalar.activation(eA[:], sA[:], EXP)
                        nc.scalar.activation(eB[:], sB[:], EXP)
                        if prev is not None:
                            ctx_mms(*prev)
                        prev = (eA, eB, kt)
                    ctx_mms(*prev)
                    # evict raw ctx + S quickly, then normalize from SBUF
                    for ab in range(2):
                        for c in range(2):
                            cps = ctxs[ab][c]
                            rc = rp.tile([P, 512], F32, tag="rc")
                            with nc.allow_low_precision(reason="f32r round"):
                                nc.vector.reciprocal(r(rc[64:65, :]),
                                                     cps[64:65, :])
                            cu = cup.tile([P, 512], F32, tag="cu")
                            nc.vector.tensor_copy(cu[:64, :], cps[0:64, :])
                            bc = ps_s.tile([P, 1024], F32, tag="s")
                            nc.tensor.matmul(bc[:64, :512],
                                             r(ones[64:65, :]),
                                             r(rc[64:65, :]),
                                             start=True, stop=True)
                            rb = rp.tile([P, 512], F32, tag="rb")
                            nc.vector.tensor_copy(rb[:64, :], bc[:64, :512])
                            gsl = slice(c * 512, (c + 1) * 512)
                            if ab == 0:
                                nc.vector.tensor_mul(
                                    r(ctx_t[qg][0:64, j, gsl]),
                                    cu[:64, :], rb[:64, :])
                            else:
                                tmpB = rp.tile([P, 512], F32, tag="tmpB")
                                nc.vector.tensor_mul(tmpB[:64, :],
                                                     cu[:64, :], rb[:64, :])
                                nc.sync.dma_start(
                                    r(ctx_t[qg][64:128, j, gsl]),
                                    r(tmpB[:64, :]))

        # ---- phase 4: output projection (partial over this head group) ----
        if 4 not in phases:
            pass
        else:
         with tc.tile_pool(name="pso", bufs=4, space="PSUM") as ps_o, \
             tc.tile_pool(name="osb", bufs=4) as op:
            for qt in range(NQT):
                for dc in range(2):
                    o_ps = ps_o.tile([P, 512], F32, tag="o")
                    # warm-keeper (discarded; real j=0 matmul below restarts
                    # the accumulation) keeps TensorE dense across the
                    # eviction gap
                    nc.tensor.matmul(
                        o_ps[:],
                        r(ctx_t[qt // 8][:, 0, (qt % 8) * P:(qt % 8 + 1) * P]),
                        r(wo_sb[:, 0, dc * 512:(dc + 1) * 512]),
                        start=True, stop=True)
                    for j in range(4):
                        nc.tensor.matmul(
                            o_ps[:],
                            r(ctx_t[qt // 8][:, j,
                                    (qt % 8) * P:(qt % 8 + 1) * P]),
                            r(wo_sb[:, j, dc * 512:(dc + 1) * 512]),
                            start=(j == 0), stop=(j == 3))
                    o_sb = op.tile([P, 512], F32, tag="osb")
                    nc.vector.tensor_copy(o_sb[:], o_ps[:])
                    nc.sync.dma_start(
                        out[qt * P:(qt + 1) * P, dc * 512:(dc + 1) * 512],
                        o_sb[:])


def build(loop_k: int = 1, phases=(1, 2, 3, 4)):
    nc = bacc.Bacc("TRN2", target_bir_lowering=False, debug=False)
    io = {
        "xT": nc.dram_tensor("xT", [D, T], BF16, kind="ExternalInput").ap(),
        "wq": nc.dram_tensor("wq", [D, G], BF16, kind="ExternalInput").ap(),
        "wk": nc.dram_tensor("wk", [D, G], BF16, kind="ExternalInput").ap(),
        "wv": nc.dram_tensor("wv", [D, VW], BF16, kind="ExternalInput").ap(),
        "wo": nc.dram_tensor("wo", [G, D], F32, kind="ExternalInput").ap(),
        "bq": nc.dram_tensor("bq", [P, 4], F32, kind="ExternalInput").ap(),
        "bk": nc.dram_tensor("bk", [P, 4], F32, kind="ExternalInput").ap(),
        "bv": nc.dram_tensor("bv", [P, VW], F32, kind="ExternalInput").ap(),
        "ones": nc.dram_tensor("ones", [P, 64], F32, kind="ExternalInput").ap(),
        "out": nc.dram_tensor("out", [T, D], F32, kind="ExternalOutput").ap(),
    }
    with tile.TileContext(nc) as tc:
        if loop_k == 1:
            emit_body(tc, io, phases)
        else:
            with tc.For_i(0, loop_k, 1):
                emit_body(tc, io, phases)
    nc.compile()
    return nc


def prep_inputs(x, Wq, bq, Wk, bk, Wv, bv, Wo, bo):
    """Host-side sharding: returns in_maps for cores 0..7."""
    f = np.float32
    bf = ml_dtypes.bfloat16
    in_maps = []
    for c in range(8):
        b, g = c // 2, c % 2
        gs = slice(g * G, (g + 1) * G)
        wv_aug = np.zeros((D, VW), f)
        bv_aug = np.zeros((VW,), f)
        wv_g = np.ascontiguousarray(Wv[gs, :].T)        # [D, 512]
        for h in range(GH):
            wv_aug[:, h * 65:h * 65 + 64] = wv_g[:, h * 64:(h + 1) * 64]
            bv_aug[h * 65:h * 65 + 64] = bv[gs][h * 64:(h + 1) * 64]
            bv_aug[h * 65 + 64] = 1.0
        in_maps.append({
            "xT": np.ascontiguousarray(np.asarray(x[b]).T).astype(bf),
            "wq": np.ascontiguousarray(Wq[gs, :].T).astype(bf),
            "wk": np.ascontiguousarray(Wk[gs, :].T).astype(bf),
            "wv": wv_aug.astype(bf),
            "wo": np.ascontiguousarray(Wo[:, gs].T),
            "bq": np.ascontiguousarray(bq[gs].reshape(4, P).T),
            "bk": np.ascontiguousarray(bk[gs].reshape(4, P).T),
            "bv": np.broadcast_to(bv_aug, (P, VW)).copy(),
            "ones": np.ones((P, 64), f),
        })
    return in_maps


def gather_output(results, bo):
    out = np.empty((B, T, D), np.float32)
    for b in range(B):
        out[b] = (results[2 * b]["out"] + results[2 * b + 1]["out"]
                  + np.asarray(bo)[None, :])
    return out


_nc_cache = {}


def kernel(x, Wq, bq, Wk, bk, Wv, bv, Wo, bo):
    if "nc" not in _nc_cache:
        _nc_cache["nc"] = build()
    nc = _nc_cache["nc"]
    in_maps = prep_inputs(x, Wq, bq, Wk, bk, Wv, bv, Wo, bo)
    res = run_bass_kernel_spmd(nc, in_maps, list(range(8)))
    return gather_output(res.results, bo)



# revision 45
# speedup vs baseline: 1.3533x; 1.3533x over previous
"""Multi-head self-attention (B=4, T=2048, D=1024, H=16) on 8 NeuronCores.

Sharding: batch x head-group. Core c handles batch b = c//2 and head group
g = c%2 (8 heads of 64 dims each). Host pre-transposes x and slices/
transposes the weights; each core computes its 8 heads' attention and a
partial output projection; host sums the two partials per batch and adds bo.

Fully-fused single-pass structure (v2). All matmuls bf16 (fp8 fails the
2e-2 tolerance: measured 1.8e-2 for fp8 e/v alone). Key design points:

  - x and all weights SBUF-resident; x streamed per-dk interleaved with
    wq/wk chunks so the dk-outer prologue projection computes during the
    load instead of waiting for all 4MB.
  - attention runs in 16 chunks (q-chunk x head-pair), 512 q at a time;
    per k-tile: 2 score matmuls (heads row-tiled at partitions 0/64), one
    [128,1024] exp covering both heads, 2 ctx matmuls with 1-iteration
    skew so the in-order PE queue never waits on ScalarE.
  - ~19% of exp tiles run on the otherwise-idle VectorE via the bit-trick
    fast exp (round(A*x+B) bitcast to f32, ~3% max err) to relieve the
    ScalarE bottleneck (HW exp is (N+352)/1.2GHz =~ 294us/core for all
    tiles).
  - qk/v projections for later pairs and the out-projection are emitted as
    fine-grained "filler" sub-units (<=0.9us: the scores->exp lead is only
    2 k-tiles deep, so bigger fillers stall ScalarE) inside earlier
    chunks' k-loops; empty slots get a discarded warm-keeper matmul so
    HAM never down-clocks the PE.
  - chunk order (qc,j): 3 qc's per pair-sweep then qc 3, spreading both
    projection deadlines and out-proj availability across the timeline.
  - softmax normalization is split and deferred into the next chunk: one
    PSUM->SBUF eviction at kt 0 (frees the ctx accumulator fast), the
    reciprocal + ones-matmul partition-broadcast + multiplies at kt 2.
  - ctx/wo/out in bf16 (f32r is 2 cyc/row on HW; bf16 partials also halve
    the output DMA), host sums the two per-batch partials in f32.
"""

from contextlib import ExitStack

import numpy as np
import ml_dtypes

import concourse.bass as bass
import concourse.mybir as mybir
import concourse.tile as tile
from concourse import bacc
from concourse.bass_utils import run_bass_kernel_spmd

F32 = mybir.dt.float32
F32R = mybir.dt.float32r
BF16 = mybir.dt.bfloat16
I32 = mybir.dt.int32
EXP = mybir.ActivationFunctionType.Exp

# bit-trick fast exp: exp(x) ~= bitcast_f32(round(A*x + B)); ~3% max rel
# err. Runs on the (otherwise idle) VectorE to offload the ScalarE exp
# bottleneck for a subset of k-tiles.
FEXP_A = float(2**23 / np.log(2.0))
FEXP_B = float(127.0 * 2**23 - 0.043677448 * 2**23)
FEXP_KTS = (4, 9, 13)

B, T, D = 4, 2048, 1024
H, DH = 16, 64
G = 512            # head-group width (8 heads x 64)
GH = 8             # heads per group
P = 128
DK = D // P        # 8 contraction k-tiles for D
NKT = T // P       # 16 key tiles of 128
NQC = 4            # q chunks of 512
QW = 512           # q width per attention chunk
VW = GH * (DH + 1)  # 520: v free width incl. ones columns
PW = 2 * (DH + 1)  # 130: v width per head pair


def r(ap):
    return ap.bitcast(F32R)


def emit_body(tc, io, phases=(1, 2, 3, 4)):
    nc = tc.nc
    xT, wq, wk, wv, wo, bq, bk, bv, out = (
        io["xT"], io["wq"], io["wk"], io["wv"], io["wo"],
        io["bq"], io["bk"], io["bv"], io["out"])

    with ExitStack() as ctx:
        E = ctx.enter_context
        constp = E(tc.tile_pool(name="const", bufs=1))
        wp = E(tc.tile_pool(name="wsb", bufs=1))
        xp = E(tc.tile_pool(name="xsb", bufs=1))
        vp = E(tc.tile_pool(name="vsb", bufs=1))
        qkp = E(tc.tile_pool(name="qksb", bufs=1))
        ctxp = E(tc.tile_pool(name="ctxsb", bufs=1))
        ep = E(tc.tile_pool(name="e", bufs=6))
        fep = E(tc.tile_pool(name="fexp", bufs=2))
        rp = E(tc.tile_pool(name="recip", bufs=2))
        op = E(tc.tile_pool(name="osb", bufs=4))
        ps_s = E(tc.tile_pool(name="pss", bufs=2, space="PSUM"))
        ps_ctx = E(tc.tile_pool(name="psctx", bufs=1, space="PSUM"))
        ps_f = E(tc.tile_pool(name="psfill", bufs=2, space="PSUM"))

        # ---- constants / weights resident in SBUF ----
        # DMA order matters: biases first (gate DVE evictions), then per-dk
        # [wq, wk, x] trios so the dk-outer prologue projection starts on
        # dk 0 while later chunks stream in; wv mid-stream, wo last.
        ones = constp.tile([P, 64], F32, name="ones")
        nc.sync.dma_start(r(ones[:]), r(io["ones"]))
        bq_sb = constp.tile([P, 4], F32, name="bq_sb")
        nc.sync.dma_start(bq_sb[:], bq[:])
        bk_sb = constp.tile([P, 4], F32, name="bk_sb")
        nc.sync.dma_start(bk_sb[:], bk[:])
        bv_sb = constp.tile([P, VW], F32, name="bv_sb")

        wq_sb = wp.tile([P, DK, G], BF16, name="wq_sb")
        wk_sb = wp.tile([P, DK, G], BF16, name="wk_sb")
        wv_sb = wp.tile([P, DK, VW], BF16, name="wv_sb")
        wo_sb = wp.tile([P, 4, D], BF16, name="wo_sb")
        x_sb = xp.tile([P, DK, T], BF16, name="x_sb")
        wq_r = wq.rearrange("(i p) m -> p i m", p=P)
        wk_r = wk.rearrange("(i p) m -> p i m", p=P)
        xT_r = xT.rearrange("(i p) t -> p i t", p=P)
        # stream only pair 0's wq/wk columns inside the trios (the x chunk
        # is the long pole; narrow weight slices keep the stream PE-bound);
        # pairs 1-3 follow right after x, well before the p2 fillers run
        for dk in range(DK):
            nc.sync.dma_start(wq_sb[:, dk, 0:P], wq_r[:, dk, 0:P])
            nc.sync.dma_start(wk_sb[:, dk, 0:P], wk_r[:, dk, 0:P])
            nc.sync.dma_start(x_sb[:, dk, :], xT_r[:, dk, :])
            if dk == 3:
                nc.sync.dma_start(
                    wv_sb[:], wv.rearrange("(i p) m -> p i m", p=P))
                nc.sync.dma_start(bv_sb[:], bv[:])
        nc.sync.dma_start(wq_sb[:, :, P:G], wq_r[:, :, P:G])
        nc.sync.dma_start(wk_sb[:, :, P:G], wk_r[:, :, P:G])
        nc.sync.dma_start(wo_sb[:], wo.rearrange("(i p) m -> p i m", p=P))

        v_sb = vp.tile([P, NKT, VW], BF16, name="v_sb")
        qT_t = [qkp.tile([P, T], BF16, name=f"qT_sb{j}") for j in range(4)]
        kT_t = [qkp.tile([P, T], BF16, name=f"kT_sb{j}") for j in range(4)]
        ctx_sb = ctxp.tile([P, 4, T], BF16, name="ctx_sb")

        # ---- filler units (emitted into attention chunks' PE gaps) ----
        # Units are sized ~0.4-0.9us: the scores->exp lead is only ~2
        # k-tiles deep (s pool bufs=2), so any filler bigger than the
        # per-kt PE slack stalls ScalarE on HW. p2 is split into 4
        # sub-units of 2 dk each; the subs of one (j, qc) group must pop
        # consecutively (they share the fill-pool psum slots).
        def p2_subs(j, qc):
            qs = slice(qc * QW, (qc + 1) * QW)
            state = {}

            def sub(dk):
                def emit():
                    if dk == 0:
                        state["q"] = ps_f.tile([P, QW], F32, tag="fill",
                                               name=f"qps{j}_{qc}")
                        state["k"] = ps_f.tile([P, QW], F32, tag="fill",
                                               name=f"kps{j}_{qc}")
                    q_ps, k_ps = state["q"], state["k"]
                    nc.tensor.matmul(
                        q_ps[:], wq_sb[:, dk, j * P:(j + 1) * P],
                        x_sb[:, dk, qs],
                        start=(dk == 0), stop=(dk == DK - 1))
                    nc.tensor.matmul(
                        k_ps[:], wk_sb[:, dk, j * P:(j + 1) * P],
                        x_sb[:, dk, qs],
                        start=(dk == 0), stop=(dk == DK - 1))
                    if dk == DK - 1:
                        # scores scale 1/sqrt(dh)=1/8 folded into q so exp
                        # runs with scale=1
                        nc.vector.tensor_scalar(
                            qT_t[j][:, qs], q_ps[:], bq_sb[:, j:j + 1],
                            0.125, mybir.AluOpType.add, mybir.AluOpType.mult)
                        nc.vector.tensor_scalar_add(
                            kT_t[j][:, qs], k_ps[:], bk_sb[:, j:j + 1])
                return emit

            return [sub(dk) for dk in range(DK)]

        # v filler covers pairs 2+3 in one N=260 matmul per dk so the
        # per-dk LDWEIGHTS (~107ns) hides under the matmul (~108ns); the
        # prologue covers pairs 0+1 the same way.
        def v_unit(ti, c0, c1):
            def emit():
                v_ps = ps_f.tile([P, QW], F32, tag="fill", name=f"vps{ti}_{c0}")
                w = c1 - c0
                for dk in range(DK):
                    nc.tensor.matmul(
                        v_ps[:, :w], x_sb[:, dk, ti * P:(ti + 1) * P],
                        wv_sb[:, dk, c0:c1], start=(dk == 0),
                        stop=(dk == DK - 1))
                nc.vector.tensor_add(
                    v_sb[:, ti, c0:c1], v_ps[:, :w], bv_sb[:, c0:c1])
            return emit

        def o_subs(qt, dc):
            state = {}

            def sub(j0):
                def emit():
                    if j0 == 0:
                        state["o"] = ps_f.tile([P, QW], F32, tag="fill",
                                               name=f"ops{qt}_{dc}")
                    o_ps = state["o"]
                    for j in (j0, j0 + 1):
                        nc.tensor.matmul(
                            o_ps[:], ctx_sb[:, j, qt * P:(qt + 1) * P],
                            wo_sb[:, j, dc * 512:(dc + 1) * 512],
                            start=(j == 0), stop=(j == 3))
                    if j0 == 2:
                        o_sb = op.tile([P, 512], BF16, tag="osb")
                        nc.vector.tensor_copy(o_sb[:], o_ps[:])
                        nc.sync.dma_start(
                            out[qt * P:(qt + 1) * P,
                                dc * 512:(dc + 1) * 512], o_sb[:])
                return emit

            return [sub(0), sub(2)]

        def o_unit(qt, dc):
            subs = o_subs(qt, dc)

            def emit():
                for s_fn in subs:
                    s_fn()
            return emit

        do_v = 1 in phases
        do_p2 = 2 in phases
        do_att = 3 in phases
        do_out = 4 in phases

        if not do_p2:
            for j in range(4):
                nc.vector.memset(qT_t[j][:], 0.0)
                nc.vector.memset(kT_t[j][:], 0.0)
        if not do_v:
            nc.vector.memset(v_sb[:], 0.0)
        if not do_att:
            nc.vector.memset(ctx_sb[:], 0.0)

        # prologue: pair 0's projections (attention chunk 0 needs them).
        # dk-outer so each x chunk's arrival unlocks 8 matmuls — PE works
        # during the x DMA stream instead of waiting for all of it. PSUM
        # tiles borrowed from the attention pools (idle until chunk 0).
        if do_p2:
            sA = ps_s.tile([P, 2 * QW], F32, tag="s", name="pro_sA")
            sB = ps_s.tile([P, 2 * QW], F32, tag="s", name="pro_sB")
            proK = ps_ctx.tile([P, 2 * QW], F32, tag="ctx", name="pro_k")
            f0 = ps_f.tile([P, QW], F32, tag="fill", name="pro_f0")
            f1 = ps_f.tile([P, QW], F32, tag="fill", name="pro_f1")
            qps = [sA[:, 0:QW], sA[:, QW:2 * QW], sB[:, 0:QW], sB[:, QW:2 * QW]]
            kps = [proK[:, 0:QW], proK[:, QW:2 * QW], f0[:], f1[:]]
            for dk in range(DK):
                for qc in range(NQC):
                    qs = slice(qc * QW, (qc + 1) * QW)
                    nc.tensor.matmul(
                        qps[qc], wq_sb[:, dk, 0:P], x_sb[:, dk, qs],
                        start=(dk == 0), stop=(dk == DK - 1))
                    nc.tensor.matmul(
                        kps[qc], wk_sb[:, dk, 0:P], x_sb[:, dk, qs],
                        start=(dk == 0), stop=(dk == DK - 1))
            for qc in range(NQC):
                qs = slice(qc * QW, (qc + 1) * QW)
                nc.vector.tensor_scalar(
                    qT_t[0][:, qs], qps[qc], bq_sb[:, 0:1], 0.125,
                    mybir.AluOpType.add, mybir.AluOpType.mult)
                nc.vector.tensor_scalar_add(
                    kT_t[0][:, qs], kps[qc], bk_sb[:, 0:1])
        if do_v:
            for ti in range(NKT):
                v_unit(ti, 0, 2 * PW)()

        # filler queue: (min_chunk_idx, emit_fn); chunk idx = qc*4 + j
        fillers = []
        # chunk sequence: 3 qc's per pair-sweep, then qc 3 — spreads both
        # the p2/v deadlines (pair j first used at position 3j) and the
        # out-proj availability (qc 0/1/2 complete at positions 9/10/11)
        seq = [(qc, j) for j in range(4) for qc in range(3)] + \
              [(3, j) for j in range(4)]
        last_pos = {qc: max(i for i, (q, _) in enumerate(seq) if q == qc)
                    for qc in range(NQC)}
        # pair j's qT/kT first used at chunk position 3j; v pairs 2+3 (one
        # unit covers both) by position 6. Spread each group's units over
        # its window; list order is the pop order (mins non-decreasing)
        # and p2 sub-groups stay contiguous (they share fill-pool slots).
        if do_p2:
            for j in range(1, 4):
                lo, hi = 3 * (j - 1), 3 * j
                for i, qc in enumerate(range(NQC)):
                    mn = lo + i * (hi - lo) // NQC
                    for u in p2_subs(j, qc):
                        fillers.append((mn, u))
        if do_v:
            for i, ti in enumerate(range(NKT)):
                fillers.append((i * 6 // NKT, v_unit(ti, 2 * PW, 4 * PW)))
            fillers.sort(key=lambda t: t[0])
        if do_out and do_att:
            for qc in range(NQC - 1):
                for qt in range(4):
                    for dc in range(2):
                        for u in o_subs(qc * 4 + qt, dc):
                            fillers.append((last_pos[qc] + 2, u))
        fillers.sort(key=lambda t: t[0])

        def pop_filler(chunk_idx):
            for i, (mn, fn) in enumerate(fillers):
                if mn <= chunk_idx:
                    fillers.pop(i)
                    return fn
            return None

        # ---- fused attention (+ interleaved fillers) ----
        # Emission order per k-tile: scores, exp, [deferred norm], filler,
        # ctx(kt-1). Fillers sit between scores and the exp-gated ctx
        # matmuls so the in-order PE queue never head-blocks on ScalarE.
        # Each chunk's normalization is deferred into the next chunk so the
        # recip->bcast chain hides behind that chunk's first scores.
        if do_att:
            pending_a = pending_b = None
            since_fill = 99
            for chunk, (qc, j) in enumerate(seq):
                    qs = slice(qc * QW, (qc + 1) * QW)
                    ca = j * PW
                    cb = j * PW + (DH + 1)
                    ctx2 = ps_ctx.tile([P, 2 * QW], F32, tag="ctx",
                                       name=f"ctx2_{qc}_{j}")
                    prevs = []

                    def ctx_mms(e, kt, ctx2=ctx2, ca=ca, cb=cb):
                        nc.tensor.matmul(
                            ctx2[:DH + 1, 0:QW], v_sb[:, kt, ca:ca + DH + 1],
                            e[:, 0:QW], start=(kt == 0), stop=(kt == NKT - 1))
                        nc.tensor.matmul(
                            ctx2[:DH + 1, QW:2 * QW],
                            v_sb[:, kt, cb:cb + DH + 1],
                            e[:, QW:2 * QW],
                            start=(kt == 0), stop=(kt == NKT - 1))

                    # norm part a (next chunk's kt 0): one copy evicts the
                    # raw ctx+S from PSUM (frees the slot for that chunk's
                    # ctx accumulation) and the reciprocals run off-PSUM
                    def norm_a(ctx2=ctx2, qc=qc, j=j):
                        cu = rp.tile([P, 2 * QW], F32, tag="cu",
                                     name=f"cu{qc}_{j}")
                        nc.vector.tensor_copy(cu[:DH + 1, :],
                                              ctx2[:DH + 1, :])
                        rc = rp.tile([P, 2 * QW], F32, tag="rc")
                        with nc.allow_low_precision(reason="f32r round"):
                            nc.vector.reciprocal(r(rc[64:65, :]),
                                                 cu[DH:DH + 1, :])
                        return cu, rc

                    # norm part b (next chunk's kt 2): ones-matmul partition
                    # broadcast of 1/S, multiply; head B lands on partitions
                    # 64-127 of ctx_sb via SBUF-SBUF DMA
                    def norm_b(cu_rc, j=j, qs=qs):
                        cu, rc = cu_rc
                        bc = ps_s.tile([P, 2 * QW], F32, tag="s",
                                       name=f"bc{j}_{qs.start}")
                        nc.tensor.matmul(bc[0:64, 0:QW], r(ones[64:65, :]),
                                         r(rc[64:65, 0:QW]),
                                         start=True, stop=True,
                                         tile_position=(64, 0))
                        nc.tensor.matmul(bc[0:64, QW:2 * QW],
                                         r(ones[64:65, :]),
                                         r(rc[64:65, QW:2 * QW]),
                                         start=True, stop=True,
                                         tile_position=(64, 0))
                        nc.vector.tensor_mul(
                            ctx_sb[0:64, j, qs], cu[0:64, 0:QW],
                            bc[0:64, 0:QW])
                        tmpB = rp.tile([P, QW], BF16, tag="tmpB")
                        nc.vector.tensor_mul(tmpB[0:64, :],
                                             cu[0:64, QW:2 * QW],
                                             bc[0:64, QW:2 * QW])
                        nc.sync.dma_start(ctx_sb[64:128, j, qs], tmpB[0:64, :])

                    for kt in range(NKT):
                        ks = slice(kt * P, (kt + 1) * P)
                        s = ps_s.tile([P, 2 * QW], F32, tag="s")
                        fn = pop_filler(chunk)
                        since_fill = 0 if fn is not None else since_fill + 1
                        if fn is None and since_fill >= 7:
                            # warm-keeper: discarded (the real scores matmul
                            # below also has start=True), keeps TensorE dense
                            # through the exp-wait gap so HAM never throttles.
                            # Suppressed for 2 slots after each filler so the
                            # scores->exp lead recovers at 295ns/kt instead
                            # of 82ns/kt.
                            nc.tensor.matmul(
                                s[:, 0:QW], kT_t[j][0:64, ks],
                                qT_t[j][0:64, qs],
                                start=True, stop=True, tile_position=(0, 0))
                        nc.tensor.matmul(
                            s[:, 0:QW], kT_t[j][0:64, ks], qT_t[j][0:64, qs],
                            start=True, stop=True, tile_position=(0, 0))
                        nc.tensor.matmul(
                            s[:, QW:2 * QW], kT_t[j][64:128, ks],
                            qT_t[j][64:128, qs],
                            start=True, stop=True, tile_position=(64, 0))
                        e = ep.tile([P, 2 * QW], BF16, tag="e")
                        if kt in FEXP_KTS:
                            ti = fep.tile([P, 2 * QW], I32, tag="fe")
                            nc.vector.tensor_scalar(
                                ti[:], s[:], FEXP_A, FEXP_B,
                                mybir.AluOpType.mult, mybir.AluOpType.add)
                            nc.vector.tensor_copy(e[:], ti.bitcast(F32)[:])
                        else:
                            nc.scalar.activation(e[:], s[:], EXP)
                        if kt == 0 and pending_a is not None:
                            pending_b = pending_b_fn(pending_a())
                            pending_a = None
                        if fn is not None:
                            fn()
                        if kt == 2 and pending_b is not None:
                            pending_b()
                            pending_b = None
                        # 2-iteration skew: ctx for kt-2 — a full extra exp
                        # period of elasticity so filler-induced PE deficits
                        # never starve ScalarE
                        if len(prevs) == 2:
                            ctx_mms(*prevs.pop(0))
                        prevs.append((e, kt))
                    for pv in prevs:
                        ctx_mms(*pv)
                    pending_a = norm_a

                    def pending_b_fn(cu_rc, norm_b=norm_b):
                        return lambda: norm_b(cu_rc)
            norm_b(pending_a())
            # tail warm-keepers: cover the norm DVE chain + ctx_sb DMA gap
            # before the final out-projection units reach the PE
            sw = ps_s.tile([P, 2 * QW], F32, tag="s", name="tail_warm")
            for _ in range(8):
                nc.tensor.matmul(
                    sw[:, 0:QW], kT_t[3][0:64, 0:P],
                    qT_t[3][0:64, 3 * QW:4 * QW],
                    start=True, stop=True, tile_position=(0, 0))

        # drain remaining fillers (p2/v if attention disabled, last out-proj)
        for _, fn in fillers:
            fn()
        if do_out and do_att:
            for qt in range(12, 16):
                for dc in range(2):
                    o_unit(qt, dc)()
        elif do_out:
            for qt in range(16):
                for dc in range(2):
                    o_unit(qt, dc)()


def build(loop_k: int = 1, phases=(1, 2, 3, 4)):
    nc = bacc.Bacc("TRN2", target_bir_lowering=False, debug=False)
    io = {
        "xT": nc.dram_tensor("xT", [D, T], BF16, kind="ExternalInput").ap(),
        "wq": nc.dram_tensor("wq", [D, G], BF16, kind="ExternalInput").ap(),
        "wk": nc.dram_tensor("wk", [D, G], BF16, kind="ExternalInput").ap(),
        "wv": nc.dram_tensor("wv", [D, VW], BF16, kind="ExternalInput").ap(),
        "wo": nc.dram_tensor("wo", [G, D], BF16, kind="ExternalInput").ap(),
        "bq": nc.dram_tensor("bq", [P, 4], F32, kind="ExternalInput").ap(),
        "bk": nc.dram_tensor("bk", [P, 4], F32, kind="ExternalInput").ap(),
        "bv": nc.dram_tensor("bv", [P, VW], F32, kind="ExternalInput").ap(),
        "ones": nc.dram_tensor("ones", [P, 64], F32, kind="ExternalInput").ap(),
        "out": nc.dram_tensor("out", [T, D], BF16, kind="ExternalOutput").ap(),
    }
    with tile.TileContext(nc) as tc:
        if loop_k == 1:
            emit_body(tc, io, phases)
        else:
            with tc.For_i(0, loop_k, 1):
                emit_body(tc, io, phases)
    nc.compile()
    return nc


def prep_inputs(x, Wq, bq, Wk, bk, Wv, bv, Wo, bo):
    """Host-side sharding: returns in_maps for cores 0..7."""
    f = np.float32
    bf = ml_dtypes.bfloat16
    in_maps = []
    for c in range(8):
        b, g = c // 2, c % 2
        gs = slice(g * G, (g + 1) * G)
        wv_aug = np.zeros((D, VW), f)
        bv_aug = np.zeros((VW,), f)
        wv_g = np.ascontiguousarray(Wv[gs, :].T)        # [D, 512]
        for h in range(GH):
            wv_aug[:, h * 65:h * 65 + 64] = wv_g[:, h * 64:(h + 1) * 64]
            bv_aug[h * 65:h * 65 + 64] = bv[gs][h * 64:(h + 1) * 64]
            bv_aug[h * 65 + 64] = 1.0
        in_maps.append({
            "xT": np.ascontiguousarray(np.asarray(x[b]).T).astype(bf),
            "wq": np.ascontiguousarray(Wq[gs, :].T).astype(bf),
            "wk": np.ascontiguousarray(Wk[gs, :].T).astype(bf),
            "wv": wv_aug.astype(bf),
            "wo": np.ascontiguousarray(Wo[:, gs].T).astype(bf),
            "bq": np.ascontiguousarray(bq[gs].reshape(4, P).T),
            "bk": np.ascontiguousarray(bk[gs].reshape(4, P).T),
            "bv": np.broadcast_to(bv_aug, (P, VW)).copy(),
            "ones": np.ones((P, 64), f),
        })
    return in_maps


def gather_output(results, bo):
    out = np.empty((B, T, D), np.float32)
    for b in range(B):
        out[b] = (results[2 * b]["out"].astype(np.float32)
                  + results[2 * b + 1]["out"].astype(np.float32)
                  + np.asarray(bo)[None, :])
    return out


_nc_cache = {}


def kernel(x, Wq, bq, Wk, bk, Wv, bv, Wo, bo):
    if "nc" not in _nc_cache:
        _nc_cache["nc"] = build()
    nc = _nc_cache["nc"]
    in_maps = prep_inputs(x, Wq, bq, Wk, bk, Wv, bv, Wo, bo)
    res = run_bass_kernel_spmd(nc, in_maps, list(range(8)))
    return gather_output(res.results, bo)
